# revision 20
# baseline (speedup 1.0000x reference)
"""Trainium2 Bass kernel for the masked-attention block (nn_MAB_61607010894006).

Sharding: data-parallel over batch B=8 across 8 NeuronCores (one batch row
per core, weights replicated, no collectives).

Per-core strategy: activations live transposed ("feature-major",
[features, tokens]); all matmul operands are float16 (full PE rate; the
fp32/fp32r path runs in fp32_mode=HIGH at half clock). PSUM accumulation is
fp32 throughout, evictions round to fp16.

  qT/kT      = W.T @ X.T      (lhsT = W chunk, rhs = XT chunk)
  S^T        = kT_h' @ qT_h   (k tokens on partitions, q tokens free)
  softmax    : exp on ScalarE with mask as per-partition bias (-1e9), no
               max-subtraction (scores are O(1)); normalization deferred:
  o^T        = [v | 1]' @ A^T accumulated over k tiles; the ones column is
               placed last for even heads / first for odd heads so the AV
               output lands on partitions 0..63 / 64..127 directly and the
               denominator on row 64 / 63; reciprocal on DVE
               (reciprocal_approx_fast), PE-broadcast, normalize+residual.
  layernorm  : feature-dim (partition) sums via ones-column matmuls on PE;
               per-token stats broadcast back via tiny PE ones-matmuls.
  FC         = Wo' @ OT, bias+relu fused into the ScalarE eviction.

Mask compaction: only unmasked key tokens are shipped per core (masked ones
contribute exactly +0.0 to the softmax numerator and denominator), padded
to a 128 multiple.
"""

import sys

sys.path.insert(0, "/opt/trn_rl_repo")

import numpy as np

import concourse.bass as bass
import concourse.mybir as mybir
import concourse.tile as tile
from concourse.bass_utils import run_bass_kernel_spmd

F32 = mybir.dt.float32
F16 = mybir.dt.float16
AF = mybir.ActivationFunctionType

B, NQ, NK, D, H, DH = 8, 1024, 1024, 512, 8, 64
EPS = 1e-5
NEG = -1e9
N_CORES = 8

MM = F16


def _split_multi_waits(nc):
    """This toolchain's walrus allows ONE sem wait per TPB instruction; Tile
    can emit several (kernel-tail drain). Hoist extras onto preceding
    single-wait NOPs on the same engine stream (equivalent: in-order issue).
    """
    multi_update = []
    for fn in nc.m.functions:
        for bb in fn.blocks:
            insts = bb.instructions
            new = []
            changed = False
            for inst in insts:
                si = inst.sync_info
                if si is not None and si.on_wait and len(si.on_wait) > 1:
                    waits = list(si.on_wait)
                    for w in waits[:-1]:
                        nop = mybir.InstNoOp(
                            name=f"I-wsplit-{nc.next_id()}", engine=inst.engine
                        )
                        nop.sync_info = mybir.SyncInfo(on_wait=[w], on_update=[])
                        new.append(nop)
                    inst.sync_info = mybir.SyncInfo(
                        on_wait=[waits[-1]], on_update=list(si.on_update)
                    )
                    changed = True
                if si is not None and si.on_update and len(si.on_update) > 1:
                    multi_update.append(inst.name)
                new.append(inst)
            if changed:
                bb.instructions = new
    if multi_update:
        raise RuntimeError(f">1 sem update unsupported: {multi_update[:10]}")


def _act_raw(nc, out, in_, func, bias=0.0, scale=1.0):
    """Raw InstActivation (bypasses the bass Reciprocal/Rsqrt accuracy guard;
    measured ~4e-5 max rel err on HW, inside this kernel's error budget)."""
    eng = nc.scalar
    inputs = [eng.lower_ap(in_)]
    for arg in (bias, scale, 0.0):
        inputs.append(mybir.ImmediateValue(dtype=mybir.dt.float32, value=arg))
    return eng.add_instruction(
        mybir.InstActivation(
            name=f"I-actraw-{nc.next_id()}",
            func=func,
            ins=inputs,
            outs=[eng.lower_ap(out)],
        )
    )


def chunks(n, w=512):
    out, s = [], 0
    while s < n:
        out.append((s, min(w, n - s)))
        s += min(w, n - s)
    return out


def build_nc(kt_tiles=8):
    NKP = kt_tiles * 128  # compacted+padded key/value token count
    nc = bass.Bass()

    qt_d = nc.dram_tensor("qt", [D, NQ], MM, kind="ExternalInput")
    kt_d = nc.dram_tensor("kt", [D + 1, NKP], MM, kind="ExternalInput")  # +ones
    wq_d = nc.dram_tensor("wq", [D, D], MM, kind="ExternalInput")
    wk_d = nc.dram_tensor("wk", [D, D], MM, kind="ExternalInput")
    wv_d = nc.dram_tensor("wv", [D + 1, D], MM, kind="ExternalInput")  # +bv row
    wo_d = nc.dram_tensor("wo", [D, D], MM, kind="ExternalInput")
    bq_d = nc.dram_tensor("bq", [128, 4], F32, kind="ExternalInput")
    bk_d = nc.dram_tensor("bk", [128, 4], F32, kind="ExternalInput")
    bo_d = nc.dram_tensor("bo", [128, 4], F32, kind="ExternalInput")
    mb_d = nc.dram_tensor("mb", [128, kt_tiles], F32, kind="ExternalInput")
    gb_d = nc.dram_tensor("gb", [128, 16], F32, kind="ExternalInput")  # g0b0g1b1
    on_d = nc.dram_tensor("on", [128, 128], MM, kind="ExternalInput")  # all ones
    out_d = nc.dram_tensor("out", [D, NQ], F32, kind="ExternalOutput")

    mult, add = mybir.AluOpType.mult, mybir.AluOpType.add

    with tile.TileContext(nc) as tc:
        with (
            tc.tile_pool(name="wp", bufs=1) as wp,
            tc.tile_pool(name="ap", bufs=1) as ap,
            tc.tile_pool(name="sm", bufs=2) as sm,
            tc.tile_pool(name="pp", bufs=2, space="PSUM") as pp,
        ):
            # ---- weights (one batched DMA each; wo deferred to the end of
            # the issue stream since it is only needed at phase 4) ----------
            wq_sb = wp.tile([128, 4 * D], MM, name="wq_sb")
            wk_sb = wp.tile([128, 4 * D], MM, name="wk_sb")
            wv_sb = wp.tile([128, 4 * D], MM, name="wv_sb")
            wv1_sb = wp.tile([1, D], MM, name="wv1_sb")
            wo_sb = wp.tile([128, 4 * D], MM, name="wo_sb")
            bq_sb = wp.tile([128, 4], F32, name="bq_sb")
            bk_sb = wp.tile([128, 4], F32, name="bk_sb")
            bo_sb = wp.tile([128, 4], F32, name="bo_sb")
            mb_sb = wp.tile([128, kt_tiles], F32, name="mb_sb")
            gb_sb = wp.tile([128, 16], F32, name="gb_sb")
            ones_sb = wp.tile([128, 128], MM, name="ones_sb")
            kt1_sb = wp.tile([1, NKP], MM, name="kt1_sb")

            def load4(dst, src, n):
                # dst [128, 4*n] <- src [4*128, n] as one strided DMA
                nc.sync.dma_start(
                    dst.rearrange("p (t n) -> p t n", t=4),
                    src.rearrange("(t p) n -> p t n", p=128),
                )

            # ---- staging (released after phase 1) ----------------------------
            with tc.tile_pool(name="stg", bufs=1) as stg:
                qt_sb = stg.tile([128, 4 * NQ], MM, name="qt_sb")
                kt_sb = stg.tile([128, 4 * NKP], MM, name="kt_sb")

                # spread loads over several engine DMA queues so transfers
                # run on parallel DMA engines; qt/kt arrive per-chunk so the
                # first projection accumulation starts as early as possible
                nc.sync.dma_start(
                    wq_sb.rearrange("p (t n) -> p t n", t=4),
                    wq_d.rearrange("(t p) n -> p t n", p=128),
                )
                for kc in range(4):
                    nc.scalar.dma_start(
                        qt_sb[:, kc * NQ : (kc + 1) * NQ],
                        qt_d[kc * 128 : (kc + 1) * 128, :],
                    )
                nc.gpsimd.dma_start(
                    wk_sb.rearrange("p (t n) -> p t n", t=4),
                    wk_d.rearrange("(t p) n -> p t n", p=128),
                )
                nc.sync.dma_start(bq_sb[:], bq_d[:])
                nc.sync.dma_start(bk_sb[:], bk_d[:])
                for kc in range(4):
                    nc.gpsimd.dma_start(
                        kt_sb[:, kc * NKP : (kc + 1) * NKP],
                        kt_d[kc * 128 : (kc + 1) * 128, :],
                    )
                nc.scalar.dma_start(
                    wv_sb.rearrange("p (t n) -> p t n", t=4),
                    wv_d[0:D, :].rearrange("(t p) n -> p t n", p=128),
                )
                nc.sync.dma_start(wv1_sb[:, :], wv_d[D : D + 1, :])
                nc.sync.dma_start(kt1_sb[:, :], kt_d[D : D + 1, :])
                nc.sync.dma_start(mb_sb[:], mb_d[:])
                nc.sync.dma_start(ones_sb[:], on_d[:])
                nc.sync.dma_start(gb_sb[:], gb_d[:])
                nc.sync.dma_start(bo_sb[:], bo_d[:])
                nc.sync.dma_start(
                    wo_sb.rearrange("p (t n) -> p t n", t=4),
                    wo_d.rearrange("(t p) n -> p t n", p=128),
                )

                ones128 = ones_sb[:, 0:1]
                ones_f32 = wp.tile([128, 1], F32, name="ones_f32")
                nc.vector.memset(ones_f32[:], 1.0)

                # ---- persistent activations ------------------------------
                q_sb = ap.tile([128, 4 * NQ], MM, name="q_sb")
                k_sb = ap.tile([128, 4 * NKP], MM, name="k_sb", tag="kmm_sq")
                # v: per k-tile, 8 heads of [v(64)|1]
                v_sb = ap.tile([128, kt_tiles * 520], MM, name="v_sb")
                v_ones = v_sb.rearrange(
                    "p (i hh x) -> p i hh x", i=kt_tiles, hh=8
                )[:, :, :, 64]
                nc.vector.memset(v_ones, 1.0)

                # ---- phase 1: projections --------------------------------
                for t in range(4):
                    for cs, cw in chunks(NQ):
                        ps_q = pp.tile([128, 512], F32, name="ps_q", tag="pp")
                        for kc in range(4):
                            nc.tensor.matmul(
                                ps_q[:, 0:cw],
                                wq_sb[:, kc * D + t * 128 : kc * D + (t + 1) * 128],
                                qt_sb[:, kc * NQ + cs : kc * NQ + cs + cw],
                                start=(kc == 0),
                                stop=(kc == 3),
                            )
                        dst = slice(t * NQ + cs, t * NQ + cs + cw)
                        nc.scalar.activation(
                            q_sb[:, dst], ps_q[:, 0:cw], AF.Identity,
                            bias=bq_sb[:, t : t + 1],
                        )
                    for cs, cw in chunks(NKP):
                        ps_k = pp.tile([128, 512], F32, name="ps_k", tag="pp")
                        for kc in range(4):
                            nc.tensor.matmul(
                                ps_k[:, 0:cw],
                                wk_sb[:, kc * D + t * 128 : kc * D + (t + 1) * 128],
                                kt_sb[:, kc * NKP + cs : kc * NKP + cs + cw],
                                start=(kc == 0),
                                stop=(kc == 3),
                            )
                        dst = slice(t * NKP + cs, t * NKP + cs + cw)
                        nc.scalar.activation(
                            k_sb[:, dst], ps_k[:, 0:cw], AF.Identity,
                            bias=bk_sb[:, t : t + 1],
                        )

                # v token-major [NKP, 512] (+bias via augmented ones row)
                for vt in range(kt_tiles):
                    ps_v = pp.tile([128, 512], F32, name="ps_v", tag="pp")
                    for kc in range(4):
                        nc.tensor.matmul(
                            ps_v[:],
                            kt_sb[:, kc * NKP + vt * 128 : kc * NKP + (vt + 1) * 128],
                            wv_sb[:, kc * D : (kc + 1) * D],
                            start=(kc == 0),
                            stop=False,
                        )
                    nc.tensor.matmul(
                        ps_v[:],
                        kt1_sb[0:1, vt * 128 : (vt + 1) * 128],
                        wv1_sb[0:1, :],
                        start=False,
                        stop=True,
                    )
                    v_blk = v_sb[:, vt * 520 : (vt + 1) * 520].rearrange(
                        "p (hh x) -> p hh x", hh=8
                    )
                    s_blk = ps_v.rearrange("p (hh x) -> p hh x", hh=8)
                    nc.scalar.copy(v_blk[:, :, 0:64], s_blk[:, :, :])

            # ---- phase 2: attention ------------------------------------------
            # Per-(head, token) softmax denominator: reciprocal on the ACT
            # table (in place at partition 64, so partition bases match),
            # then PE-broadcast across the 64 head features.
            o_sb = ap.tile([128, 4 * NQ], MM, name="o_sb", tag="big", bufs=2)
            # base-0 copy of q lanes 64..127 for odd heads (engine ops cannot
            # mix partition bases; DMA can move across partitions)
            q_lo = ap.tile([64, 4 * NQ], MM, name="q_lo")
            nc.gpsimd.dma_start(q_lo[:], q_sb[64:128, :])

            def head_normalize(h, po_t, rinvs):
                # emitted AFTER the next head's score matmuls so the PE does
                # not stall waiting for the ACT-table reciprocal
                pr, rh = h // 2, (h % 2) * 64
                for c in range(2):
                    po, rinv = po_t[c], rinvs[c]
                    pb = pp.tile([64, 512], F32, name="pb", tag="pp")
                    nc.tensor.matmul(
                        pb[:], ones_sb[64:65, 0:64], rinv[64:65, :],
                        start=True, stop=True,
                    )
                    rb = sm.tile([64, 512], MM, name="rb", tag="rb")
                    nc.vector.tensor_copy(rb[:], pb[:])
                    avn = sm.tile([64, 512], MM, name="avn", tag="avn")
                    nc.vector.tensor_mul(avn[:, :], po[0:64], rb[:])
                    qsl = slice(pr * NQ + c * 512, pr * NQ + (c + 1) * 512)
                    if rh == 0:
                        nc.vector.tensor_add(
                            o_sb[0:64, qsl], avn[:, :], q_sb[0:64, qsl]
                        )
                    else:
                        # odd head: build at base 0, then shift to lanes
                        # 64..127 via SBUF-to-SBUF DMA on the Pool queue
                        opre = sm.tile([64, 512], MM, name="opre", tag="opre")
                        nc.vector.tensor_add(opre[:, :], avn[:, :], q_lo[:, qsl])
                        nc.gpsimd.dma_start(o_sb[64:128, qsl], opre[:, :])

            pending = None
            for h in range(H):
                pr, rh, odd = h // 2, (h % 2) * 64, h % 2
                at_tiles = []
                for i in range(kt_tiles):
                    ps_s = pp.tile([128, NQ], F32, name="ps_s", tag="ps")
                    for c in range(2):
                        nc.tensor.matmul(
                            ps_s[:, c * 512 : (c + 1) * 512],
                            k_sb[rh : rh + 64,
                                 pr * NKP + i * 128 : pr * NKP + (i + 1) * 128],
                            q_sb[rh : rh + 64,
                                 pr * NQ + c * 512 : pr * NQ + (c + 1) * 512],
                            start=True,
                            stop=True,
                        )
                    at_sb = ap.tile([128, NQ], MM, name="at_sb", tag="at", bufs=8)
                    at_tiles.append(at_sb)
                    nc.scalar.activation(
                        at_sb[:, :], ps_s[:, :], AF.Exp,
                        bias=mb_sb[:, i : i + 1], scale=0.125,
                    )
                    if pending is not None and i == min(1, kt_tiles - 1):
                        head_normalize(*pending)
                        pending = None
                if pending is not None:
                    head_normalize(*pending)
                    pending = None
                po_t = []
                for c in range(2):
                    po = pp.tile([65, 512], F32, name="po", tag="po")
                    po_t.append(po)
                    for i in range(kt_tiles):
                        nc.tensor.matmul(
                            po[:],
                            v_sb[:, i * 520 + h * 65 : i * 520 + (h + 1) * 65],
                            at_tiles[i][:, c * 512 : (c + 1) * 512],
                            start=(i == 0),
                            stop=(i == kt_tiles - 1),
                        )
                rinvs = []
                for c in range(2):
                    po = po_t[c]
                    rinv = sm.tile([65, 512], MM, name="rinv", tag="rinv")
                    rinvs.append(rinv)
                    _act_raw(nc, rinv[64:65, :], po[64:65, :], AF.Reciprocal)
                pending = (h, po_t, rinvs)
            head_normalize(*pending)

            # ---- layernorm helper --------------------------------------------
            def layer_norm(x_sb, gcol, bcol, out_sb, dma_out=None):
                ones_x = ones_f32 if x_sb.dtype == F32 else ones128
                sq = ap.tile([128, 4 * NQ], MM, name="sq", tag="kmm_sq")
                for t in range(4):
                    sl = slice(t * NQ, (t + 1) * NQ)
                    nc.vector.tensor_mul(sq[:, sl], x_sb[:, sl], x_sb[:, sl])
                mu = sm.tile([1, NQ], F32, name="mu", tag="mu", bufs=1)
                ex2 = sm.tile([1, NQ], F32, name="ex2", tag="ex2", bufs=1)
                for c in range(2):
                    ps_su = pp.tile([1, 512], F32, name="ps_su", tag="po")
                    ps_sq = pp.tile([1, 512], F32, name="ps_sq", tag="po")
                    for t in range(4):
                        sl = slice(t * NQ + c * 512, t * NQ + (c + 1) * 512)
                        nc.tensor.matmul(
                            ps_su[:], ones_x, x_sb[:, sl],
                            start=(t == 0), stop=(t == 3),
                        )
                        nc.tensor.matmul(
                            ps_sq[:], ones128, sq[:, sl],
                            start=(t == 0), stop=(t == 3),
                        )
                    csl = slice(c * 512, (c + 1) * 512)
                    nc.scalar.activation(
                        mu[:, csl], ps_su[:], AF.Identity, scale=1.0 / D
                    )
                    nc.scalar.activation(
                        ex2[:, csl], ps_sq[:], AF.Identity, scale=1.0 / D
                    )
                var = sm.tile([1, NQ], F32, name="var", tag="var", bufs=1)
                nc.vector.tensor_mul(var[:], mu[:], mu[:])
                nc.vector.tensor_sub(var[:], ex2[:], var[:])
                rstd = sm.tile([1, NQ], F32, name="rstd", tag="rstd", bufs=1)
                _act_raw(nc, rstd[:], var[:], AF.Rsqrt, bias=EPS)
                rstd_h = sm.tile([1, NQ], MM, name="rstd_h", tag="rstdh", bufs=1)
                mur_h = sm.tile([1, NQ], MM, name="mur_h", tag="murh", bufs=1)
                nc.vector.tensor_copy(rstd_h[:], rstd[:])
                nc.vector.tensor_mul(mur_h[:], mu[:], rstd[:])
                for c in range(2):
                    csl = slice(c * 512, (c + 1) * 512)
                    pb1 = pp.tile([128, 512], F32, name="pb1", tag="pp")
                    nc.tensor.matmul(
                        pb1[:], ones_sb[0:1, :], rstd_h[0:1, csl],
                        start=True, stop=True,
                    )
                    pb2 = pp.tile([128, 512], F32, name="pb2", tag="pp")
                    nc.tensor.matmul(
                        pb2[:], ones_sb[0:1, :], mur_h[0:1, csl],
                        start=True, stop=True,
                    )
                    for t in range(4):
                        sl = slice(t * NQ + c * 512, t * NQ + (c + 1) * 512)
                        nc.vector.tensor_mul(out_sb[:, sl], x_sb[:, sl], pb1[:])
                        nc.vector.tensor_sub(out_sb[:, sl], out_sb[:, sl], pb2[:])
                        nc.vector.tensor_scalar(
                            out_sb[:, sl], out_sb[:, sl],
                            gb_sb[:, gcol + t : gcol + t + 1],
                            gb_sb[:, bcol + t : bcol + t + 1],
                            mult, add,
                        )
                        if dma_out is not None:
                            nc.sync.dma_start(
                                dma_out[t * 128 : (t + 1) * 128,
                                        c * 512 : (c + 1) * 512],
                                out_sb[:, sl],
                            )

            # ---- phase 3: LN0 -------------------------------------------------
            ot0 = ap.tile([128, 4 * NQ], MM, name="ot0", tag="big", bufs=2)
            layer_norm(o_sb, 0, 4, ot0)

            # ---- phase 4: FC + relu + residual -------------------------------
            o1 = ap.tile([128, 4 * NQ], MM, name="o1", tag="big", bufs=2)
            for ot in range(4):
                for c in range(2):
                    ps_f = pp.tile([128, 512], F32, name="ps_f", tag="pp")
                    for ft in range(4):
                        nc.tensor.matmul(
                            ps_f[:],
                            wo_sb[:, ft * D + ot * 128 : ft * D + (ot + 1) * 128],
                            ot0[:, ft * NQ + c * 512 : ft * NQ + (c + 1) * 512],
                            start=(ft == 0),
                            stop=(ft == 3),
                        )
                    rl = sm.tile([128, 512], MM, name="rl", tag="avn")
                    nc.scalar.activation(
                        rl[:], ps_f[:], AF.Relu, bias=bo_sb[:, ot : ot + 1],
                    )
                    sl = slice(ot * NQ + c * 512, ot * NQ + (c + 1) * 512)
                    nc.vector.tensor_add(o1[:, sl], ot0[:, sl], rl[:])

            # ---- phase 5: LN1 -> out ------------------------------------------
            otout = ap.tile([128, 4 * NQ], F32, name="otout", tag="bigo", bufs=1)
            layer_norm(o1, 8, 12, otout, dma_out=out_d)

    _split_multi_waits(nc)
    return nc


_nc_cache = {}


def _get_nc(kt_tiles=8):
    if kt_tiles not in _nc_cache:
        _nc_cache[kt_tiles] = build_nc(kt_tiles)
    return _nc_cache[kt_tiles]


def _kt_tiles_for(mask):
    n = int(max(int((mask[b] != 0).sum()) for b in range(mask.shape[0])))
    return max(1, (n + 127) // 128)


def prep_inputs(Q, K, mask, Wq, bq, Wk, bk, Wv, bv, Wo, bo, g0, b0, g1, b1,
                kt_tiles=None):
    f32 = np.float32
    f16 = np.float16
    ones_h = np.ones((128, 128), f16)
    if kt_tiles is None:
        kt_tiles = _kt_tiles_for(mask)
    nkp = kt_tiles * 128

    def percol(v, dt=f32):  # [512] feature vector -> [128, 4] per-partition
        return np.ascontiguousarray(np.asarray(v, f32).reshape(4, 128).T.astype(dt))

    wv_h = np.ascontiguousarray(
        np.vstack([np.asarray(Wv, f32), np.asarray(bv, f32)[None, :]]).astype(f16)
    )
    gb = np.concatenate([percol(g0), percol(b0), percol(g1), percol(b1)], axis=1)
    wq_h = np.ascontiguousarray(np.asarray(Wq, f32).astype(f16))
    wk_h = np.ascontiguousarray(np.asarray(Wk, f32).astype(f16))
    wo_h = np.ascontiguousarray(np.asarray(Wo, f32).astype(f16))

    in_maps = []
    for b in range(B):
        qt = np.ascontiguousarray(np.asarray(Q[b], f32).T.astype(f16))
        idx = np.nonzero(mask[b] != 0)[0]
        kc = np.zeros((nkp, D), f32)
        kc[: len(idx)] = np.asarray(K[b], f32)[idx]
        kt = np.ascontiguousarray(
            np.vstack([kc.T, np.ones((1, nkp), f32)]).astype(f16)
        )
        mb = np.full(nkp, np.float32(NEG))
        mb[: len(idx)] = 0.0
        mb = np.ascontiguousarray(mb.reshape(kt_tiles, 128).T.astype(f32))
        in_maps.append(
            {
                "qt": qt,
                "kt": kt,
                "wq": wq_h,
                "wk": wk_h,
                "wv": wv_h,
                "wo": wo_h,
                "bq": percol(bq),
                "bk": percol(bk),
                "bo": percol(bo),
                "mb": mb,
                "gb": gb,
                "on": ones_h,
            }
        )
    return in_maps


def kernel(Q, K, mask, Wq, bq, Wk, bk, Wv, bv, Wo, bo, g0, b0, g1, b1):
    mask = np.asarray(mask)
    kt_tiles = _kt_tiles_for(mask)
    nc = _get_nc(kt_tiles)
    in_maps = prep_inputs(
        Q, K, mask, Wq, bq, Wk, bk, Wv, bv, Wo, bo, g0, b0, g1, b1, kt_tiles
    )
    res = run_bass_kernel_spmd(nc, in_maps, list(range(N_CORES)))
    out = np.stack(
        [np.ascontiguousarray(res.results[i]["out"].T) for i in range(N_CORES)]
    )
    return out.astype(np.float32)
